# revision 43
# baseline (speedup 1.0000x reference)
"""Distributed Trainium2 attention-block kernel (8 NeuronCores).

Problem: y = LN(x) -> QKV -> 16-head attention (seq 2048, dh 64) -> out-proj.
x [2,2048,1024] f32.

Sharding: token-parallel. Core c handles batch c//4, token quarter c%4
(512 query tokens). Each core computes Q,K,V for its own 512 tokens
(all heads), publishes K^T and augmented V via TWO 8-core AllGathers
with Shared outputs (RDH algorithm, ~42us each; the naive four sub-1MB
4-core-group gathers run Mesh at ~30us each but serialize to ~130us).
AG#1 carries K/V for head-group 0 (head pairs 0-3), AG#2 for head-group
1, matching consumption order. The collective init barrier (~35-60us,
runtime-controlled) plus ~11us of firmware latency floors the first AG
start at ~60-75us regardless of trigger time; LN + QKV + the local
attention passes cover that shadow. Remote chunks DMA at rank-dependent
offsets from the Shared gather buffer: such dynamic-offset reads only
lower as a single AP per snapped base (~3 base registers per DMA
queue), so each sibling chunk is fetched as one combined K+V [1024,520]
read, g0 on the sync queue, g1 on gpsimd.

All operands bf16. Softmax probability jitter transfers ~1:1 to the
output (it does NOT average away), so the absolute score error must
stay <~0.01 - that rules out fp8 anywhere (q/k/v/p chain AND both
projections); bf16 keeps total rel err ~7e-3 vs the 2e-2 gate.

Attention per head: dots computed transposed (k on partitions, q free)
with two heads sharing the PE via 64-row tile_position groups (the two
tiles stream concurrently at full row rate); exp'd probabilities (bf16)
feed PV as the moving operand; PV's stationary is [V_tile | ones]
(M=65) so the softmax denominator accumulates in PSUM row 64 for free.
Softmax skips the max-subtraction: scaled dots are ~N(0,1) by
construction. ScalarE's exp (~128us at 1 elem/cycle/lane) saturates
during attention and co-paces the PE, so norm work is pushed off DVE:
head-pair normalization overlaps the following pass, with the all-SBUF
broadcast-multiplies routed through the otherwise-idle GpSimd.

Startup: the four x-tile loads are issued before the 8MB weight-slab
prefetch (which otherwise starves them at HBM and stalls LayerNorm for
~20us). Tail: the output projection is split it=0..6 / it=7 so the last
head pair's normalization chain hides under the first 56 projection
matmuls.
"""

import os
import numpy as np

import concourse.bass as bass
import concourse.tile as tile
from concourse import mybir
from concourse.bass import ds
from concourse.bass_utils import run_bass_kernel_spmd
from concourse.masks import make_identity

F32 = mybir.dt.float32
BF16 = mybir.dt.bfloat16

B, S, D = 2, 2048, 1024
H, DH = 16, 64
T = 512           # query tokens per core
P = 128
NKT = S // P      # 16 k-tiles
LN_EPS = 1e-5
SCALE = DH ** -0.5
EXP_BATCH = 2     # k-tiles per exp ACTIVATE call

_MAXW = 1


def _split_multiwaits(nc):
    """This container's walrus rejects >1 sync wait/update per instruction.
    Move extras onto adjacent same-engine NoOps."""
    import bass_rust

    for bb in nc.main_func.blocks:
        new_insts = []
        for inst in bb.instructions:
            si = inst.sync_info
            pre, post = [], []
            if si is not None:
                waits = list(si.on_wait or [])
                ups = list(si.on_update or [])
                if len(waits) > _MAXW or len(ups) > _MAXW:
                    for i in range(_MAXW, len(waits), _MAXW):
                        pre.append(bass_rust.InstNoOp(
                            name=f"I-{nc.next_id()}", engine=inst.engine,
                            ins=[], outs=[],
                            sync_info=mybir.SyncInfo(
                                on_wait=waits[i:i + _MAXW], on_update=[])))
                    for i in range(_MAXW, len(ups), _MAXW):
                        post.append(bass_rust.InstNoOp(
                            name=f"I-{nc.next_id()}", engine=inst.engine,
                            ins=[], outs=[],
                            sync_info=mybir.SyncInfo(
                                on_wait=[], on_update=ups[i:i + _MAXW])))
                    inst.sync_info = mybir.SyncInfo(
                        on_wait=waits[:_MAXW], on_update=ups[:_MAXW])
            new_insts.extend(pre)
            new_insts.append(inst)
            new_insts.extend(post)
        bb.instructions[:] = new_insts


def _maybe_install_ntff_hook():
    """Optional NTFF profiling support (BASS_TRACE=1); harmless if absent."""
    if not os.environ.get("BASS_TRACE"):
        return
    import sys
    import types
    if "antenv.axon_hooks" in sys.modules:
        return
    try:
        mod = types.ModuleType("antenv.axon_hooks")
        _h = [None]
        mod.set_axon_ntff_profile_hook = lambda h: _h.__setitem__(0, h)
        mod.get_axon_ntff_profile_hook = lambda: _h[0]
        import antenv
        from trn_agent_boot.trn_boot import _ntff_profile_via_ctypes
        hook = _ntff_profile_via_ctypes('/opt/axon/libaxon_pjrt.so')
        sys.modules["antenv.axon_hooks"] = mod
        antenv.axon_hooks = mod
        mod.set_axon_ntff_profile_hook(hook)
    except Exception:
        pass


def build(apply_ln_affine, apply_b_out):
    nc = bass.Bass()

    x_ext = nc.declare_dram_parameter("x", [T, D], BF16, isOutput=False)
    gamma_ext = nc.declare_dram_parameter("ln_gamma", [1, D], F32, isOutput=False)
    beta_ext = nc.declare_dram_parameter("ln_beta", [1, D], F32, isOutput=False)
    wqkv_ext = nc.declare_dram_parameter("w_qkv16", [D, 3 * D], BF16,
                                         isOutput=False)
    wout_ext = nc.declare_dram_parameter("w_o16", [D, D], BF16, isOutput=False)
    bout_ext = nc.declare_dram_parameter("b_out", [1, D], F32, isOutput=False)
    out_ext = nc.declare_dram_parameter("out", [T, D], BF16, isOutput=True)

    g8 = [[0, 1, 2, 3, 4, 5, 6, 7]]
    NDT = D // P   # 8 contraction tiles over model dim
    NTT = T // P   # 4 token tiles per core
    NHP = H // 2   # 8 head pairs
    VA = 2 * 65    # augmented-v columns per head pair

    from contextlib import ExitStack
    with tile.TileContext(nc) as tc, ExitStack() as stack:
        consts = stack.enter_context(tc.tile_pool(name="consts", bufs=1))
        sb_main = stack.enter_context(tc.tile_pool(name="sb_main", bufs=1))
        p23 = stack.enter_context(tc.tile_pool(name="p23", bufs=1))

        eps_t = consts.tile([P, 1], F32)
        nc.vector.memset(eps_t, LN_EPS)
        ones8 = consts.tile([P, 8], F32)
        nc.vector.memset(ones8, 1.0)
        # identity for PE transposes, built on gpsimd BEFORE its queue fills
        # with the weight-slab DMA triggers
        ident = consts.tile([P, P], BF16)
        make_identity(nc, ident)

        # w16 slab pool opened before the x pool (pools release LIFO; the
        # x pool closes after phase 1, the slab after phase 2)
        wpool_cm = tc.tile_pool(name="wpool", bufs=1)
        wpool = wpool_cm.__enter__()
        w16 = wpool.tile([P, NDT, 3 * D], BF16, tag="w16", name="w16")

        # x tiles loaded FIRST (1MB) so the 8MB weight prefetch can't starve
        # the LayerNorm critical path at HBM
        xpool_cm = tc.tile_pool(name="xp", bufs=1)
        xpool = xpool_cm.__enter__()
        x_tiles = [xpool.tile([P, D], BF16, tag=f"x{tt}", name=f"x{tt}")
                   for tt in range(NTT)]
        for tt in range(NTT):
            nc.sync.dma_start(out=x_tiles[tt],
                              in_=x_ext[tt * P:(tt + 1) * P, :])

        if apply_ln_affine:
            gammaB = consts.tile([P, D], F32)
            betaB = consts.tile([P, D], F32)
            nc.sync.dma_start(out=gammaB, in_=bass.AP(
                tensor=gamma_ext.tensor, offset=gamma_ext.offset,
                ap=[[0, P]] + gamma_ext.ap[1:]))
            nc.sync.dma_start(out=betaB, in_=bass.AP(
                tensor=beta_ext.tensor, offset=beta_ext.offset,
                ap=[[0, P]] + beta_ext.ap[1:]))
        if apply_b_out:
            boutB = consts.tile([P, D], F32)
            nc.sync.dma_start(out=boutB, in_=bass.AP(
                tensor=bout_ext.tensor, offset=bout_ext.offset,
                ap=[[0, P]] + bout_ext.ap[1:]))

        # persistent activations
        xnT = [sb_main.tile([P, T], BF16, tag=f"xnT{i}", name=f"xnT{i}")
               for i in range(NDT)]
        qT = [sb_main.tile([P, T], BF16, tag=f"qT{i}", name=f"qT{i}")
              for i in range(NHP)]
        attnT = [sb_main.tile([P, T], BF16, tag=f"attnT{i}", name=f"attnT{i}")
                 for i in range(NHP)]
        wout_sb = sb_main.tile([P, NDT, D], BF16, tag="wout", name="wout")
        # local K^T / augmented-V (this core's token chunk), kept resident
        kt_l = [p23.tile([P, T], BF16, tag=f"ktl{i}", name=f"ktl{i}")
                for i in range(NHP)]                  # i = 4*g + hq
        v_l = [p23.tile([P, 4 * VA], BF16, tag=f"vl{i}", name=f"vl{i}")
               for i in range(8)]                     # i = 4*g + token tile

        # AllGather buffers. kv_in[g] rows 0-511: K^T feature-half g
        # (512 feat x 512 own tokens, cols 0:512); rows 512-1023: augmented
        # V for head-group g (512 own tokens x 520). Gathered over all 8
        # cores into a Shared buffer (RDH); cores read only their batch
        # group's 4 chunks.
        kv_in2 = [nc.dram_tensor(f"kv_in{g}", [2 * T, VA * 4], BF16).ap()
                  for g in range(2)]
        kv_out2 = [nc.dram_tensor(f"kv_out{g}", [8 * 2 * T, VA * 4], BF16,
                                  addr_space="Shared").ap()
                   for g in range(2)]
        recip_d = nc.dram_tensor("recip_d", [H, T], F32).ap()

        # ---- w16 slab DMAs (48KB/partition), freed after the projections ----
        wq_view = wqkv_ext.rearrange("(dt p) f -> dt p f", p=P)

        def wslab_dma(lo, hi):
            nc.gpsimd.dma_start(
                out=w16[:, :, lo:hi],
                in_=wq_view[:, :, lo:hi].rearrange("dt p f -> p dt f"))

        # weight slab DMAs on the gpsimd queue (never blocks the x loads),
        # in consumption order: k g0, v g0, k g1, v g1, q, w_out.
        wslab_dma(D, D + T)
        wslab_dma(2 * D, 2 * D + T)
        wslab_dma(D + T, 2 * D)
        wslab_dma(2 * D + T, 3 * D)
        wslab_dma(0, D)
        wo_view = wout_ext.rearrange("(it p) f -> it p f", p=P)
        nc.gpsimd.dma_start(
            out=wout_sb, in_=wo_view.rearrange("it p f -> p it f"))

        # ---------------- Phase 1: LayerNorm + transpose ----------------
        with tc.tile_pool(name="p1sb", bufs=3) as p1sb, \
             tc.tile_pool(name="p1ps", bufs=4, space="PSUM") as p1ps:
            for tt in range(NTT):
                x_t = x_tiles[tt]
                stats = p1sb.tile([P, 2, nc.vector.BN_STATS_DIM], F32, tag="st")
                for sg in range(2):
                    nc.vector.bn_stats(out=stats[:, sg, :],
                                       in_=x_t[:, sg * 512:(sg + 1) * 512])
                mv = p1sb.tile([P, nc.vector.BN_AGGR_DIM], F32, tag="mv")
                nc.vector.bn_aggr(out=mv, in_=stats)
                rstd = p1sb.tile([P, 1], F32, tag="rstd")
                nc.scalar.activation(out=rstd, in_=mv[:, 1:2],
                                     func=mybir.ActivationFunctionType.Sqrt,
                                     bias=eps_t, scale=1.0)
                nc.vector.reciprocal(out=rstd, in_=rstd)
                xn_t = p1sb.tile([P, D], BF16, tag="xn")
                nc.vector.tensor_scalar(
                    out=xn_t, in0=x_t, scalar1=mv[:, 0:1], scalar2=rstd,
                    op0=mybir.AluOpType.subtract, op1=mybir.AluOpType.mult)
                if apply_ln_affine:
                    nc.vector.tensor_mul(out=xn_t, in0=xn_t, in1=gammaB)
                    nc.vector.tensor_add(out=xn_t, in0=xn_t, in1=betaB)
                for dt in range(NDT):
                    ps_tr = p1ps.tile([P, P], BF16, tag="tr")
                    nc.tensor.transpose(ps_tr, xn_t[:, dt * P:(dt + 1) * P],
                                        ident)
                    # alternate drain engine: DVE carries the LN stats chain,
                    # so route half the copies through the idle ScalarE
                    if dt % 2 == 0:
                        nc.vector.tensor_copy(
                            out=xnT[dt][:, tt * P:(tt + 1) * P], in_=ps_tr)
                    else:
                        nc.scalar.copy(
                            out=xnT[dt][:, tt * P:(tt + 1) * P], in_=ps_tr)
        xpool_cm.__exit__(None, None, None)

        # ---------------- Phase 2: QKV projection + AllGathers ----------------
        with tc.tile_pool(name="p2ps", bufs=4, space="PSUM") as p2ps:

            def proj_colT_pair(col0s, dsts, post=None):
                """Two interleaved accumulation chains in different PSUM
                banks, so consecutive PE instructions can overlap instead
                of serializing on the same accumulator."""
                pss = [p2ps.tile([P, T], F32, tag="prj", name=f"prj{j}")
                       for j in range(len(col0s))]
                for dt in range(NDT):
                    for ps, col0 in zip(pss, col0s):
                        nc.tensor.matmul(ps, w16[:, dt, col0:col0 + P],
                                         xnT[dt], start=(dt == 0),
                                         stop=(dt == NDT - 1))
                for i, (ps, dst) in enumerate(zip(pss, dsts)):
                    if i % 2 == 0:
                        nc.vector.tensor_copy(out=dst, in_=ps)
                    else:
                        nc.scalar.copy(out=dst, in_=ps)
                    if post is not None:
                        post(i)

            def proj_k_group(g):
                for i in range(0, 4, 2):
                    def wr(which, g=g, i=i):
                        ii = i + which
                        nc.sync.dma_start(
                            out=kv_in2[g][ii * P:(ii + 1) * P, 0:T],
                            in_=kt_l[4 * g + ii])
                    proj_colT_pair(
                        [D + g * T + (i + j) * P for j in range(2)],
                        [kt_l[4 * g + i + j] for j in range(2)], post=wr)

            def proj_v_group(g):
                for vt0 in range(0, NTT, 2):
                    pss = [p2ps.tile([P, T], F32, tag="prj",
                                     name=f"prjv{j}") for j in range(2)]
                    for dt in range(NDT):
                        for j in range(2):
                            nc.tensor.matmul(
                                pss[j],
                                xnT[dt][:, (vt0 + j) * P:(vt0 + j + 1) * P],
                                w16[:, dt, 2 * D + g * T:2 * D + (g + 1) * T],
                                start=(dt == 0), stop=(dt == NDT - 1))
                    for j in range(2):
                        vt_i = vt0 + j
                        vl = v_l[4 * g + vt_i]
                        nc.vector.tensor_copy(
                            out=vl.rearrange("p (h f) -> p h f", h=8)
                            [:, :, 0:64],
                            in_=pss[j].rearrange("p (h f) -> p h f", h=8))
                        nc.vector.tensor_copy(
                            out=vl.rearrange("p (h f) -> p h f", h=8)
                            [:, :, 64:65],
                            in_=ones8.rearrange("p (h o) -> p h o", h=8))
                        nc.sync.dma_start(
                            out=kv_in2[g][T + vt_i * P:T + (vt_i + 1) * P, :],
                            in_=vl)

            def trigger_ag(g):
                nc.gpsimd.collective_compute(
                    "AllGather", mybir.AluOpType.bypass,
                    replica_groups=g8,
                    ins=[kv_in2[g].opt()], outs=[kv_out2[g].opt()])

            proj_k_group(0)
            proj_v_group(0)
            trigger_ag(0)
            proj_k_group(1)
            proj_v_group(1)
            trigger_ag(1)

            # remote-chunk loads queued NOW: they wait on the collectives'
            # completion sems and fire the moment the data lands, instead
            # of queueing behind the q-projection / local-attention emission.
            krem2 = {}

            def load_remote(g):
                # All K-chunk loads BEFORE all V-chunk loads (K feeds the
                # dots that precede each PV pass). The DMA queues support
                # only ~4 dynamic DRAM base registers per engine, so each
                # sibling chunk's K and V reads share one snapped base (V
                # sits at constant displacement +T rows).
                # Chunks split across BOTH queues so they drain in
                # parallel after the AG lands. The urgently-consumed j0/j1
                # ride the hardware-DGE sync queue (gpsimd's software DGE
                # builds the ~1K descriptor lines on the slow Q7); only the
                # last-consumed chunks use gpsimd. 3 dynamic bases each;
                # the sync queue's last AG2-gated load (g1j0) still fires
                # ~35us before the phase-3 norm DMAs queued behind it.
                kvrem = []
                for j in range(3):
                    if g == 0:
                        eng = nc.sync if j <= 1 else nc.gpsimd
                    else:
                        eng = nc.sync if j == 0 else nc.gpsimd
                    rank = eng.partition_id()
                    off = eng.snap(
                        (rank - rank % 4 + (rank % 4 + 1 + j) % 4) * (2 * T),
                        min_val=0, max_val=7 * 2 * T)
                    kv = p23.tile([P, 8, 4 * VA], BF16, tag=f"kvr{g}_{j}",
                                  name=f"kvr{g}_{j}")
                    eng.dma_start(
                        out=kv,
                        in_=kv_out2[g][ds(off, 2 * T), :]
                        .rearrange("(c p) t -> p c t", p=P))
                    kvrem.append(kv)
                krem2[g] = kvrem

            load_remote(0)
            load_remote(1)

            for ct in range(0, NHP, 2):
                proj_colT_pair([ct * P, (ct + 1) * P],
                               [qT[ct], qT[ct + 1]])
        wpool_cm.__exit__(None, None, None)

        # ---------------- Phase 3: attention ----------------
        rem_kts = list(range(4, NKT))
        with tc.tile_pool(name="p3sb", bufs=3) as p3sb, \
             tc.tile_pool(name="p3o", bufs=1) as p3o, \
             tc.tile_pool(name="p3pt", bufs=8) as p3pt, \
             tc.tile_pool(name="p3po", bufs=2, space="PSUM") as p3po, \
             tc.tile_pool(name="p3pd", bufs=3, space="PSUM") as p3pd:
            o_cmb = [p3o.tile([65, T], BF16, tag=f"ocmb{h}", name=f"ocmb{h}")
                     for h in range(H)]
            o_loc = [p3o.tile([65, T], BF16, tag=f"oloc{h}", name=f"oloc{h}")
                     for h in range(H)]

            def k_src(g, hq, kt):
                c, w = kt // 4, kt % 4
                if c == 0:
                    return kt_l[4 * g + hq][:, w * P:(w + 1) * P]
                return krem2[g][c - 1][:, hq, w * P:(w + 1) * P]

            def v_src(g, hq, ab, kt):
                c, w = kt // 4, kt % 4
                base = hq * VA + ab * 65
                if c == 0:
                    return v_l[4 * g + w][:, base:base + 65]
                return krem2[g][c - 1][:, 4 + w, base:base + 65]

            def attn_pass(g, hq, kts_all, drain, inject=None):
                """Pipelined dots->exp->PV over kts_all; drain(ps_o) at end.
                inject() emits independent PE work between exp batches so the
                in-order PE queue has filler while waiting on ScalarE or a
                late collective."""
                hp = 4 * g + hq
                ps_o = [p3po.tile([65, T], F32, tag="po",
                                  name=f"po{drain.__name__}{hp}_{ab}")
                        for ab in range(2)]
                batches = [kts_all[i:i + EXP_BATCH]
                           for i in range(0, len(kts_all), EXP_BATCH)]
                pending = []
                first_kt = kts_all[0]
                last_kt = kts_all[-1]

                def emit_pv(pkts, ppts, is_last):
                    for i, kt in enumerate(pkts):
                        for ab in range(2):
                            nc.tensor.matmul(
                                ps_o[ab], v_src(g, hq, ab, kt),
                                ppts[ab][:, i, :],
                                start=(kt == first_kt),
                                stop=(is_last and kt == last_kt))

                for kts in batches:
                    nb = len(kts)
                    pd = [p3pd.tile([P, EXP_BATCH, T], F32, tag="pd",
                                    name=f"pd{drain.__name__}{hp}_{kts[0]}_{ab}")
                          for ab in range(2)]
                    for i, kt in enumerate(kts):
                        for ab in range(2):
                            nc.tensor.matmul(
                                pd[ab][:, i, :],
                                k_src(g, hq, kt)[ab * 64:(ab + 1) * 64, :],
                                qT[hp][ab * 64:(ab + 1) * 64, :],
                                start=True, stop=True,
                                tile_position=(ab * 64, 0))
                    pts = []
                    for ab in range(2):
                        pt = p3pt.tile([P, EXP_BATCH, T], BF16, tag="pt")
                        nc.scalar.activation(
                            out=pt[:, 0:nb, :], in_=pd[ab][:, 0:nb, :],
                            func=mybir.ActivationFunctionType.Exp,
                            scale=SCALE)
                        pts.append(pt)
                    if inject is not None:
                        inject()
                    if len(pending) >= 1:
                        emit_pv(*pending.pop(0), False)
                    pending.append((list(kts), pts))
                while pending:
                    emit_pv(*pending.pop(0), len(pending) == 0)
                drain(hp, ps_o)

            def drain_local(hp, ps_o):
                # both on DVE: ScalarE is the exp-saturated bottleneck
                # during attention, DVE has slack here
                nc.vector.tensor_copy(out=o_loc[2 * hp], in_=ps_o[0])
                nc.vector.tensor_copy(out=o_loc[2 * hp + 1], in_=ps_o[1])

            def drain_remote(hp, ps_o):
                for ab in range(2):
                    h = 2 * hp + ab
                    nc.vector.tensor_add(out=o_cmb[h], in0=ps_o[ab],
                                         in1=o_loc[h])

            def norm_group(g):
                """Deferred normalization: one reciprocal for the group's 8
                heads, broadcast across partitions via a DRAM round-trip."""
                sums16 = p3sb.tile([8, T], BF16, tag="sums16")
                for j in range(8):
                    h = 8 * g + j
                    nc.sync.dma_start(out=sums16[j:j + 1, :],
                                      in_=o_cmb[h][64:65, :])
                sums_g = p3sb.tile([8, T], F32, tag="sums")
                nc.vector.tensor_copy(out=sums_g, in_=sums16)
                nc.vector.reciprocal(out=sums_g, in_=sums_g)
                nc.sync.dma_start(out=recip_d[8 * g:8 * g + 8, :], in_=sums_g)
                for hq in range(4):
                    hp = 4 * g + hq
                    for ab in range(2):
                        h = 2 * hp + ab
                        recipB = p3sb.tile([64, T], F32, tag="rb")
                        rd = recip_d[h:h + 1, :]
                        nc.sync.dma_start(out=recipB, in_=bass.AP(
                            tensor=rd.tensor, offset=rd.offset,
                            ap=[[0, 64]] + rd.ap[1:]))
                        # all-SBUF multiply: route via the idle GpSimd to
                        # keep DVE free for PSUM drains mid-attention
                        nc.gpsimd.tensor_mul(
                            out=attnT[hp][ab * 64:(ab + 1) * 64, :],
                            in0=o_cmb[h][0:64, :],
                            in1=recipB)

            def norm_hps(hps, scalar_recip=False, mul_eng=None):
                """Normalize one or two head-pairs (finer than norm_group so
                most of g1's normalization overlaps its last remote passes).
                scalar_recip uses ScalarE's table-based reciprocal (~1
                cycle/elem vs DVE's ~6.5) — only safe after the last exp,
                since it swaps the activation table."""
                n = len(hps)
                sums16 = p3sb.tile([2 * n, T], BF16, tag=f"s16b{n}")
                for idx, hp in enumerate(hps):
                    for ab in range(2):
                        nc.sync.dma_start(
                            out=sums16[2 * idx + ab:2 * idx + ab + 1, :],
                            in_=o_cmb[2 * hp + ab][64:65, :])
                sums4 = p3sb.tile([2 * n, T], F32, tag=f"s4{n}")
                if scalar_recip:
                    nc.scalar.activation(
                        out=sums4, in_=sums16,
                        func=mybir.ActivationFunctionType.Reciprocal,
                        scale=1.0)
                else:
                    nc.vector.tensor_copy(out=sums4, in_=sums16)
                    nc.vector.reciprocal(out=sums4, in_=sums4)
                for idx, hp in enumerate(hps):
                    nc.sync.dma_start(
                        out=recip_d[2 * hp:2 * hp + 2, :],
                        in_=sums4[2 * idx:2 * idx + 2, :])
                if mul_eng is None:
                    mul_eng = nc.vector
                for hp in hps:
                    for ab in range(2):
                        h = 2 * hp + ab
                        recipB = p3sb.tile([64, T], F32, tag="rb")
                        rd = recip_d[h:h + 1, :]
                        nc.sync.dma_start(out=recipB, in_=bass.AP(
                            tensor=rd.tensor, offset=rd.offset,
                            ap=[[0, 64]] + rd.ap[1:]))
                        mul_eng.tensor_mul(
                            out=attnT[hp][ab * 64:(ab + 1) * 64, :],
                            in0=o_cmb[h][0:64, :],
                            in1=recipB)

            for g in range(2):
                for hq in range(4):
                    attn_pass(g, hq, list(range(4)), drain_local)
            for hq in range(4):
                attn_pass(0, hq, rem_kts, drain_remote)
            for hq in range(4):
                attn_pass(1, hq, rem_kts, drain_remote)
                if hq == 0:
                    norm_group(0)
                if hq == 2:
                    norm_hps([4, 5], mul_eng=nc.gpsimd)
            norm_hps([6], mul_eng=nc.gpsimd)
            norm_hps([7])

        # ---------------- Phase 4: output projection ----------------
        # Split contraction it=0..6 / it=7: the first 56 matmuls only need
        # heads 0-13, so head pair 7's normalization chain (DVE + DMA
        # round-trip) hides under them; only the last 8 matmuls wait on it.
        with tc.tile_pool(name="p4sb", bufs=3) as p4sb, \
             tc.tile_pool(name="p4ps", bufs=1, space="PSUM") as p4ps:
            ps_y = {}
            for tt in range(NTT):
                for dc in range(2):
                    ps_y[tt, dc] = p4ps.tile([P, T], F32, tag=f"py{tt}_{dc}",
                                             name=f"py{tt}_{dc}")
            # it-major emission: consecutive matmuls hit different PSUM
            # banks, so the PE can overlap them instead of serializing on
            # one accumulator chain
            for it in range(NDT - 1):
                for tt in range(NTT):
                    for dc in range(2):
                        nc.tensor.matmul(
                            ps_y[tt, dc], attnT[it][:, tt * P:(tt + 1) * P],
                            wout_sb[:, it, dc * T:(dc + 1) * T],
                            start=(it == 0), stop=False)
            for tt in range(NTT):
                y_s = p4sb.tile([P, D], BF16, tag="y")
                for dc in range(2):
                    nc.tensor.matmul(
                        ps_y[tt, dc],
                        attnT[NDT - 1][:, tt * P:(tt + 1) * P],
                        wout_sb[:, NDT - 1, dc * T:(dc + 1) * T],
                        start=False, stop=True)
                    lo, hi = dc * T, (dc + 1) * T
                    if apply_b_out:
                        nc.vector.tensor_add(out=y_s[:, lo:hi],
                                             in0=ps_y[tt, dc],
                                             in1=boutB[:, lo:hi])
                    elif dc == 0:
                        nc.vector.tensor_copy(out=y_s[:, lo:hi],
                                              in_=ps_y[tt, dc])
                    else:
                        nc.scalar.copy(out=y_s[:, lo:hi], in_=ps_y[tt, dc])
                nc.sync.dma_start(
                    out=out_ext[tt * P:(tt + 1) * P, :], in_=y_s)

    _split_multiwaits(nc)
    return nc


_CACHE = {}
LAST_RESULTS = None


def kernel(x, ln_gamma, ln_beta, w_qkv, w_out, b_out):
    global LAST_RESULTS
    import ml_dtypes
    _maybe_install_ntff_hook()

    x = np.asarray(x, dtype=np.float32)
    ln_gamma = np.asarray(ln_gamma, dtype=np.float32).reshape(1, D)
    ln_beta = np.asarray(ln_beta, dtype=np.float32).reshape(1, D)
    w_qkv = np.asarray(w_qkv, dtype=np.float32)
    w_out = np.asarray(w_out, dtype=np.float32)
    b_out = np.asarray(b_out, dtype=np.float32).reshape(1, D)

    x16 = x.astype(ml_dtypes.bfloat16)
    w_qkv16 = np.ascontiguousarray(w_qkv.astype(ml_dtypes.bfloat16))
    w_o16 = np.ascontiguousarray(w_out.astype(ml_dtypes.bfloat16))

    apply_ln_affine = not (np.all(ln_gamma == 1.0) and np.all(ln_beta == 0.0))
    apply_b_out = not np.all(b_out == 0.0)

    key = (apply_ln_affine, apply_b_out)
    if key not in _CACHE:
        _CACHE[key] = build(*key)
    nc = _CACHE[key]

    in_maps = []
    for c in range(8):
        b, t = c // 4, c % 4
        in_maps.append({
            "x": np.ascontiguousarray(x16[b, t * T:(t + 1) * T, :]),
            "ln_gamma": ln_gamma,
            "ln_beta": ln_beta,
            "w_qkv16": w_qkv16,
            "w_o16": w_o16,
            "b_out": b_out,
        })

    trace = bool(os.environ.get("BASS_TRACE"))
    res = run_bass_kernel_spmd(nc, in_maps, core_ids=list(range(8)),
                               trace=trace)
    LAST_RESULTS = res

    y = np.empty((B, S, D), dtype=np.float32)
    for c in range(8):
        b, t = c // 4, c % 4
        y[b, t * T:(t + 1) * T, :] = np.asarray(
            res.results[c]["out"], dtype=np.float32)
    return y


# revision 44
# speedup vs baseline: 1.0461x; 1.0461x over previous
"""Distributed Trainium2 attention-block kernel (8 NeuronCores).

Problem: y = LN(x) -> QKV -> 16-head attention (seq 2048, dh 64) -> out-proj.
x [2,2048,1024] f32.

Sharding: token-parallel. Core c handles batch c//4, token quarter c%4
(512 query tokens). Each core computes Q,K,V for its own 512 tokens
(all heads), publishes K^T and augmented V via TWO 8-core AllGathers
with Shared outputs (RDH algorithm, ~42us each; the naive four sub-1MB
4-core-group gathers run Mesh at ~30us each but serialize to ~130us).
AG#1 carries K/V for head-group 0 (head pairs 0-3), AG#2 for head-group
1, matching consumption order. The collective init barrier (~35-60us,
runtime-controlled) plus ~11us of firmware latency floors the first AG
start at ~60-75us regardless of trigger time; LN + QKV + the local
attention passes cover that shadow. Remote chunks DMA at rank-dependent
offsets from the Shared gather buffer: such dynamic-offset reads only
lower as a single AP per snapped base (~3 base registers per DMA
queue), so each sibling chunk is fetched as one combined K+V [1024,520]
read, g0 on the sync queue, g1 on gpsimd.

All operands bf16. Softmax probability jitter transfers ~1:1 to the
output (it does NOT average away), so the absolute score error must
stay <~0.01 - that rules out fp8 anywhere (q/k/v/p chain AND both
projections); bf16 keeps total rel err ~7e-3 vs the 2e-2 gate.

Attention per head: dots computed transposed (k on partitions, q free)
with two heads sharing the PE via 64-row tile_position groups (the two
tiles stream concurrently at full row rate); exp'd probabilities (bf16)
feed PV as the moving operand; PV's stationary is [V_tile | ones]
(M=65) so the softmax denominator accumulates in PSUM row 64 for free.
Softmax skips the max-subtraction: scaled dots are ~N(0,1) by
construction. ScalarE's exp (~128us at 1 elem/cycle/lane) saturates
during attention and co-paces the PE, so norm work is pushed off DVE:
head-pair normalization overlaps the following pass, with the all-SBUF
broadcast-multiplies routed through the otherwise-idle GpSimd.

Startup: the four x-tile loads are issued before the 8MB weight-slab
prefetch (which otherwise starves them at HBM and stalls LayerNorm for
~20us). Tail: the output projection is split it=0..6 / it=7 so the last
head pair's normalization chain hides under the first 56 projection
matmuls.
"""

import os
import numpy as np

import concourse.bass as bass
import concourse.tile as tile
from concourse import mybir
from concourse.bass import ds
from concourse.bass_utils import run_bass_kernel_spmd
from concourse.masks import make_identity

F32 = mybir.dt.float32
BF16 = mybir.dt.bfloat16

B, S, D = 2, 2048, 1024
H, DH = 16, 64
T = 512           # query tokens per core
P = 128
NKT = S // P      # 16 k-tiles
LN_EPS = 1e-5
SCALE = DH ** -0.5
EXP_BATCH = 2     # k-tiles per exp ACTIVATE call

_MAXW = 1


def _split_multiwaits(nc):
    """This container's walrus rejects >1 sync wait/update per instruction.
    Move extras onto adjacent same-engine NoOps."""
    import bass_rust

    for bb in nc.main_func.blocks:
        new_insts = []
        for inst in bb.instructions:
            si = inst.sync_info
            pre, post = [], []
            if si is not None:
                waits = list(si.on_wait or [])
                ups = list(si.on_update or [])
                if len(waits) > _MAXW or len(ups) > _MAXW:
                    for i in range(_MAXW, len(waits), _MAXW):
                        pre.append(bass_rust.InstNoOp(
                            name=f"I-{nc.next_id()}", engine=inst.engine,
                            ins=[], outs=[],
                            sync_info=mybir.SyncInfo(
                                on_wait=waits[i:i + _MAXW], on_update=[])))
                    for i in range(_MAXW, len(ups), _MAXW):
                        post.append(bass_rust.InstNoOp(
                            name=f"I-{nc.next_id()}", engine=inst.engine,
                            ins=[], outs=[],
                            sync_info=mybir.SyncInfo(
                                on_wait=[], on_update=ups[i:i + _MAXW])))
                    inst.sync_info = mybir.SyncInfo(
                        on_wait=waits[:_MAXW], on_update=ups[:_MAXW])
            new_insts.extend(pre)
            new_insts.append(inst)
            new_insts.extend(post)
        bb.instructions[:] = new_insts


def _maybe_install_ntff_hook():
    """Optional NTFF profiling support (BASS_TRACE=1); harmless if absent."""
    if not os.environ.get("BASS_TRACE"):
        return
    import sys
    import types
    if "antenv.axon_hooks" in sys.modules:
        return
    try:
        mod = types.ModuleType("antenv.axon_hooks")
        _h = [None]
        mod.set_axon_ntff_profile_hook = lambda h: _h.__setitem__(0, h)
        mod.get_axon_ntff_profile_hook = lambda: _h[0]
        import antenv
        from trn_agent_boot.trn_boot import _ntff_profile_via_ctypes
        hook = _ntff_profile_via_ctypes('/opt/axon/libaxon_pjrt.so')
        sys.modules["antenv.axon_hooks"] = mod
        antenv.axon_hooks = mod
        mod.set_axon_ntff_profile_hook(hook)
    except Exception:
        pass


def build(apply_ln_affine, apply_b_out):
    nc = bass.Bass()

    x_ext = nc.declare_dram_parameter("x", [T, D], BF16, isOutput=False)
    gamma_ext = nc.declare_dram_parameter("ln_gamma", [1, D], F32, isOutput=False)
    beta_ext = nc.declare_dram_parameter("ln_beta", [1, D], F32, isOutput=False)
    wqkv_ext = nc.declare_dram_parameter("w_qkv16", [D, 3 * D], BF16,
                                         isOutput=False)
    wout_ext = nc.declare_dram_parameter("w_o16", [D, D], BF16, isOutput=False)
    bout_ext = nc.declare_dram_parameter("b_out", [1, D], F32, isOutput=False)
    out_ext = nc.declare_dram_parameter("out", [T, D], BF16, isOutput=True)

    g8 = [[0, 1, 2, 3, 4, 5, 6, 7]]
    NDT = D // P   # 8 contraction tiles over model dim
    NTT = T // P   # 4 token tiles per core
    NHP = H // 2   # 8 head pairs
    VA = 2 * 65    # augmented-v columns per head pair

    from contextlib import ExitStack
    with tile.TileContext(nc) as tc, ExitStack() as stack:
        consts = stack.enter_context(tc.tile_pool(name="consts", bufs=1))
        sb_main = stack.enter_context(tc.tile_pool(name="sb_main", bufs=1))
        p23 = stack.enter_context(tc.tile_pool(name="p23", bufs=1))

        eps_t = consts.tile([P, 1], F32)
        nc.vector.memset(eps_t, LN_EPS)
        ones8 = consts.tile([P, 8], F32)
        nc.vector.memset(ones8, 1.0)
        # identity for PE transposes, built on gpsimd BEFORE its queue fills
        # with the weight-slab DMA triggers
        ident = consts.tile([P, P], BF16)
        make_identity(nc, ident)

        # w16 slab pool opened before the x pool (pools release LIFO; the
        # x pool closes after phase 1, the slab after phase 2)
        wpool_cm = tc.tile_pool(name="wpool", bufs=1)
        wpool = wpool_cm.__enter__()
        w16 = wpool.tile([P, NDT, 3 * D], BF16, tag="w16", name="w16")

        # x tiles loaded FIRST (1MB) so the 8MB weight prefetch can't starve
        # the LayerNorm critical path at HBM
        xpool_cm = tc.tile_pool(name="xp", bufs=1)
        xpool = xpool_cm.__enter__()
        x_tiles = [xpool.tile([P, D], BF16, tag=f"x{tt}", name=f"x{tt}")
                   for tt in range(NTT)]
        for tt in range(NTT):
            nc.sync.dma_start(out=x_tiles[tt],
                              in_=x_ext[tt * P:(tt + 1) * P, :])

        if apply_ln_affine:
            gammaB = consts.tile([P, D], F32)
            betaB = consts.tile([P, D], F32)
            nc.sync.dma_start(out=gammaB, in_=bass.AP(
                tensor=gamma_ext.tensor, offset=gamma_ext.offset,
                ap=[[0, P]] + gamma_ext.ap[1:]))
            nc.sync.dma_start(out=betaB, in_=bass.AP(
                tensor=beta_ext.tensor, offset=beta_ext.offset,
                ap=[[0, P]] + beta_ext.ap[1:]))
        if apply_b_out:
            boutB = consts.tile([P, D], F32)
            nc.sync.dma_start(out=boutB, in_=bass.AP(
                tensor=bout_ext.tensor, offset=bout_ext.offset,
                ap=[[0, P]] + bout_ext.ap[1:]))

        # persistent activations
        xnT = [sb_main.tile([P, T], BF16, tag=f"xnT{i}", name=f"xnT{i}")
               for i in range(NDT)]
        qT = [sb_main.tile([P, T], BF16, tag=f"qT{i}", name=f"qT{i}")
              for i in range(NHP)]
        attnT = [sb_main.tile([P, T], BF16, tag=f"attnT{i}", name=f"attnT{i}")
                 for i in range(NHP)]
        wout_sb = sb_main.tile([P, NDT, D], BF16, tag="wout", name="wout")
        # local K^T / augmented-V (this core's token chunk), kept resident
        kt_l = [p23.tile([P, T], BF16, tag=f"ktl{i}", name=f"ktl{i}")
                for i in range(NHP)]                  # i = 4*g + hq
        v_l = [p23.tile([P, 4 * VA], BF16, tag=f"vl{i}", name=f"vl{i}")
               for i in range(8)]                     # i = 4*g + token tile

        # AllGather buffers. kv_in[g] rows 0-511: K^T feature-half g
        # (512 feat x 512 own tokens, cols 0:512); rows 512-1023: augmented
        # V for head-group g (512 own tokens x 520). Gathered over all 8
        # cores into a Shared buffer (RDH); cores read only their batch
        # group's 4 chunks.
        kv_in2 = [nc.dram_tensor(f"kv_in{g}", [2 * T, VA * 4], BF16).ap()
                  for g in range(2)]
        kv_out2 = [nc.dram_tensor(f"kv_out{g}", [8 * 2 * T, VA * 4], BF16,
                                  addr_space="Shared").ap()
                   for g in range(2)]
        recip_d = nc.dram_tensor("recip_d", [H, T], F32).ap()

        # ---- w16 slab DMAs (48KB/partition), freed after the projections ----
        wq_view = wqkv_ext.rearrange("(dt p) f -> dt p f", p=P)

        def wslab_dma(lo, hi):
            nc.gpsimd.dma_start(
                out=w16[:, :, lo:hi],
                in_=wq_view[:, :, lo:hi].rearrange("dt p f -> p dt f"))

        # weight slab DMAs on the gpsimd queue (never blocks the x loads),
        # in consumption order: k g0, v g0, k g1, v g1, q, w_out.
        wslab_dma(D, D + T)
        wslab_dma(2 * D, 2 * D + T)
        wslab_dma(D + T, 2 * D)
        wslab_dma(2 * D + T, 3 * D)
        wslab_dma(0, D)
        wo_view = wout_ext.rearrange("(it p) f -> it p f", p=P)
        nc.gpsimd.dma_start(
            out=wout_sb, in_=wo_view.rearrange("it p f -> p it f"))

        # ---------------- Phase 1: LayerNorm + transpose ----------------
        with tc.tile_pool(name="p1sb", bufs=3) as p1sb, \
             tc.tile_pool(name="p1ps", bufs=4, space="PSUM") as p1ps:
            for tt in range(NTT):
                x_t = x_tiles[tt]
                stats = p1sb.tile([P, 2, nc.vector.BN_STATS_DIM], F32, tag="st")
                for sg in range(2):
                    nc.vector.bn_stats(out=stats[:, sg, :],
                                       in_=x_t[:, sg * 512:(sg + 1) * 512])
                mv = p1sb.tile([P, nc.vector.BN_AGGR_DIM], F32, tag="mv")
                nc.vector.bn_aggr(out=mv, in_=stats)
                rstd = p1sb.tile([P, 1], F32, tag="rstd")
                nc.scalar.activation(out=rstd, in_=mv[:, 1:2],
                                     func=mybir.ActivationFunctionType.Sqrt,
                                     bias=eps_t, scale=1.0)
                nc.vector.reciprocal(out=rstd, in_=rstd)
                xn_t = p1sb.tile([P, D], BF16, tag="xn")
                nc.vector.tensor_scalar(
                    out=xn_t, in0=x_t, scalar1=mv[:, 0:1], scalar2=rstd,
                    op0=mybir.AluOpType.subtract, op1=mybir.AluOpType.mult)
                if apply_ln_affine:
                    nc.vector.tensor_mul(out=xn_t, in0=xn_t, in1=gammaB)
                    nc.vector.tensor_add(out=xn_t, in0=xn_t, in1=betaB)
                for dt in range(NDT):
                    ps_tr = p1ps.tile([P, P], BF16, tag="tr")
                    nc.tensor.transpose(ps_tr, xn_t[:, dt * P:(dt + 1) * P],
                                        ident)
                    # alternate drain engine: DVE carries the LN stats chain,
                    # so route half the copies through the idle ScalarE
                    if dt % 2 == 0:
                        nc.vector.tensor_copy(
                            out=xnT[dt][:, tt * P:(tt + 1) * P], in_=ps_tr)
                    else:
                        nc.scalar.copy(
                            out=xnT[dt][:, tt * P:(tt + 1) * P], in_=ps_tr)
        xpool_cm.__exit__(None, None, None)

        # ---------------- Phase 2: QKV projection + AllGathers ----------------
        with tc.tile_pool(name="p2ps", bufs=4, space="PSUM") as p2ps:

            def proj_colT_pair(col0s, dsts, post=None):
                """Two interleaved accumulation chains in different PSUM
                banks, so consecutive PE instructions can overlap instead
                of serializing on the same accumulator."""
                pss = [p2ps.tile([P, T], F32, tag="prj", name=f"prj{j}")
                       for j in range(len(col0s))]
                for dt in range(NDT):
                    for ps, col0 in zip(pss, col0s):
                        nc.tensor.matmul(ps, w16[:, dt, col0:col0 + P],
                                         xnT[dt], start=(dt == 0),
                                         stop=(dt == NDT - 1))
                for i, (ps, dst) in enumerate(zip(pss, dsts)):
                    if i % 2 == 0:
                        nc.vector.tensor_copy(out=dst, in_=ps)
                    else:
                        nc.scalar.copy(out=dst, in_=ps)
                    if post is not None:
                        post(i)

            def proj_k_group(g):
                for i in range(0, 4, 2):
                    def wr(which, g=g, i=i):
                        ii = i + which
                        nc.sync.dma_start(
                            out=kv_in2[g][ii * P:(ii + 1) * P, 0:T],
                            in_=kt_l[4 * g + ii])
                    proj_colT_pair(
                        [D + g * T + (i + j) * P for j in range(2)],
                        [kt_l[4 * g + i + j] for j in range(2)], post=wr)

            def proj_v_group(g):
                for vt0 in range(0, NTT, 2):
                    pss = [p2ps.tile([P, T], F32, tag="prj",
                                     name=f"prjv{j}") for j in range(2)]
                    for dt in range(NDT):
                        for j in range(2):
                            nc.tensor.matmul(
                                pss[j],
                                xnT[dt][:, (vt0 + j) * P:(vt0 + j + 1) * P],
                                w16[:, dt, 2 * D + g * T:2 * D + (g + 1) * T],
                                start=(dt == 0), stop=(dt == NDT - 1))
                    for j in range(2):
                        vt_i = vt0 + j
                        vl = v_l[4 * g + vt_i]
                        nc.vector.tensor_copy(
                            out=vl.rearrange("p (h f) -> p h f", h=8)
                            [:, :, 0:64],
                            in_=pss[j].rearrange("p (h f) -> p h f", h=8))
                        nc.vector.tensor_copy(
                            out=vl.rearrange("p (h f) -> p h f", h=8)
                            [:, :, 64:65],
                            in_=ones8.rearrange("p (h o) -> p h o", h=8))
                        nc.sync.dma_start(
                            out=kv_in2[g][T + vt_i * P:T + (vt_i + 1) * P, :],
                            in_=vl)

            def trigger_ag(g):
                nc.gpsimd.collective_compute(
                    "AllGather", mybir.AluOpType.bypass,
                    replica_groups=g8,
                    ins=[kv_in2[g].opt()], outs=[kv_out2[g].opt()])

            proj_k_group(0)
            proj_v_group(0)
            trigger_ag(0)
            proj_k_group(1)
            proj_v_group(1)
            trigger_ag(1)

            # remote-chunk loads queued NOW: they wait on the collectives'
            # completion sems and fire the moment the data lands, instead
            # of queueing behind the q-projection / local-attention emission.
            krem2 = {}

            def load_remote(g):
                # All K-chunk loads BEFORE all V-chunk loads (K feeds the
                # dots that precede each PV pass). The DMA queues support
                # only ~4 dynamic DRAM base registers per engine, so each
                # sibling chunk's K and V reads share one snapped base (V
                # sits at constant displacement +T rows).
                # Chunks split across BOTH queues so they drain in
                # parallel after the AG lands. The urgently-consumed j0/j1
                # ride the hardware-DGE sync queue (gpsimd's software DGE
                # builds the ~1K descriptor lines on the slow Q7); only the
                # last-consumed chunks use gpsimd. 3 dynamic bases each;
                # the sync queue's last AG2-gated load (g1j0) still fires
                # ~35us before the phase-3 norm DMAs queued behind it.
                kvrem = []
                for j in range(3):
                    if g == 0:
                        eng = nc.sync if j <= 1 else nc.gpsimd
                    else:
                        eng = nc.sync if j == 0 else nc.gpsimd
                    rank = eng.partition_id()
                    off = eng.snap(
                        (rank - rank % 4 + (rank % 4 + 1 + j) % 4) * (2 * T),
                        min_val=0, max_val=7 * 2 * T)
                    kv = p23.tile([P, 8, 4 * VA], BF16, tag=f"kvr{g}_{j}",
                                  name=f"kvr{g}_{j}")
                    eng.dma_start(
                        out=kv,
                        in_=kv_out2[g][ds(off, 2 * T), :]
                        .rearrange("(c p) t -> p c t", p=P))
                    kvrem.append(kv)
                krem2[g] = kvrem

            load_remote(0)
            load_remote(1)

            for ct in range(0, NHP, 2):
                proj_colT_pair([ct * P, (ct + 1) * P],
                               [qT[ct], qT[ct + 1]])
        wpool_cm.__exit__(None, None, None)

        # ---------------- Phase 3: attention ----------------
        rem_kts = list(range(4, NKT))
        with tc.tile_pool(name="p3sb", bufs=3) as p3sb, \
             tc.tile_pool(name="p3o", bufs=1) as p3o, \
             tc.tile_pool(name="p3pt", bufs=10) as p3pt, \
             tc.tile_pool(name="p3po", bufs=2, space="PSUM") as p3po, \
             tc.tile_pool(name="p3pd", bufs=3, space="PSUM") as p3pd:
            o_cmb = [p3o.tile([65, T], BF16, tag=f"ocmb{h}", name=f"ocmb{h}")
                     for h in range(H)]
            o_loc = [p3o.tile([65, T], BF16, tag=f"oloc{h}", name=f"oloc{h}")
                     for h in range(H)]

            def k_src(g, hq, kt):
                c, w = kt // 4, kt % 4
                if c == 0:
                    return kt_l[4 * g + hq][:, w * P:(w + 1) * P]
                return krem2[g][c - 1][:, hq, w * P:(w + 1) * P]

            def v_src(g, hq, ab, kt):
                c, w = kt // 4, kt % 4
                base = hq * VA + ab * 65
                if c == 0:
                    return v_l[4 * g + w][:, base:base + 65]
                return krem2[g][c - 1][:, 4 + w, base:base + 65]

            def attn_pass(g, hq, kts_all, drain, inject=None):
                """Pipelined dots->exp->PV over kts_all; drain(ps_o) at end.
                inject() emits independent PE work between exp batches so the
                in-order PE queue has filler while waiting on ScalarE or a
                late collective."""
                hp = 4 * g + hq
                ps_o = [p3po.tile([65, T], F32, tag="po",
                                  name=f"po{drain.__name__}{hp}_{ab}")
                        for ab in range(2)]
                batches = [kts_all[i:i + EXP_BATCH]
                           for i in range(0, len(kts_all), EXP_BATCH)]
                pending = []
                first_kt = kts_all[0]
                last_kt = kts_all[-1]

                def emit_pv(pkts, ppts, is_last):
                    for i, kt in enumerate(pkts):
                        for ab in range(2):
                            nc.tensor.matmul(
                                ps_o[ab], v_src(g, hq, ab, kt),
                                ppts[ab][:, i, :],
                                start=(kt == first_kt),
                                stop=(is_last and kt == last_kt))

                for kts in batches:
                    nb = len(kts)
                    pd = [p3pd.tile([P, EXP_BATCH, T], F32, tag="pd",
                                    name=f"pd{drain.__name__}{hp}_{kts[0]}_{ab}")
                          for ab in range(2)]
                    for i, kt in enumerate(kts):
                        for ab in range(2):
                            nc.tensor.matmul(
                                pd[ab][:, i, :],
                                k_src(g, hq, kt)[ab * 64:(ab + 1) * 64, :],
                                qT[hp][ab * 64:(ab + 1) * 64, :],
                                start=True, stop=True,
                                tile_position=(ab * 64, 0))
                    pts = []
                    for ab in range(2):
                        pt = p3pt.tile([P, EXP_BATCH, T], BF16, tag="pt")
                        nc.scalar.activation(
                            out=pt[:, 0:nb, :], in_=pd[ab][:, 0:nb, :],
                            func=mybir.ActivationFunctionType.Exp,
                            scale=SCALE)
                        pts.append(pt)
                    if inject is not None:
                        inject()
                    if len(pending) >= 1:
                        emit_pv(*pending.pop(0), False)
                    pending.append((list(kts), pts))
                while pending:
                    emit_pv(*pending.pop(0), len(pending) == 0)
                drain(hp, ps_o)

            def drain_local(hp, ps_o):
                # both on DVE: ScalarE is the exp-saturated bottleneck
                # during attention, DVE has slack here
                nc.vector.tensor_copy(out=o_loc[2 * hp], in_=ps_o[0])
                nc.vector.tensor_copy(out=o_loc[2 * hp + 1], in_=ps_o[1])

            def drain_remote(hp, ps_o):
                for ab in range(2):
                    h = 2 * hp + ab
                    nc.vector.tensor_add(out=o_cmb[h], in0=ps_o[ab],
                                         in1=o_loc[h])

            def norm_group(g):
                """Deferred normalization: one reciprocal for the group's 8
                heads, broadcast across partitions via a DRAM round-trip."""
                sums16 = p3sb.tile([8, T], BF16, tag="sums16")
                for j in range(8):
                    h = 8 * g + j
                    nc.sync.dma_start(out=sums16[j:j + 1, :],
                                      in_=o_cmb[h][64:65, :])
                sums_g = p3sb.tile([8, T], F32, tag="sums")
                nc.vector.tensor_copy(out=sums_g, in_=sums16)
                nc.vector.reciprocal(out=sums_g, in_=sums_g)
                nc.sync.dma_start(out=recip_d[8 * g:8 * g + 8, :], in_=sums_g)
                for hq in range(4):
                    hp = 4 * g + hq
                    for ab in range(2):
                        h = 2 * hp + ab
                        recipB = p3sb.tile([64, T], F32, tag="rb")
                        rd = recip_d[h:h + 1, :]
                        nc.sync.dma_start(out=recipB, in_=bass.AP(
                            tensor=rd.tensor, offset=rd.offset,
                            ap=[[0, 64]] + rd.ap[1:]))
                        # all-SBUF multiply: route via the idle GpSimd to
                        # keep DVE free for PSUM drains mid-attention
                        nc.gpsimd.tensor_mul(
                            out=attnT[hp][ab * 64:(ab + 1) * 64, :],
                            in0=o_cmb[h][0:64, :],
                            in1=recipB)

            def norm_hps(hps, scalar_recip=False, mul_eng=None):
                """Normalize one or two head-pairs (finer than norm_group so
                most of g1's normalization overlaps its last remote passes).
                scalar_recip uses ScalarE's table-based reciprocal (~1
                cycle/elem vs DVE's ~6.5) — only safe after the last exp,
                since it swaps the activation table."""
                n = len(hps)
                sums16 = p3sb.tile([2 * n, T], BF16, tag=f"s16b{n}")
                for idx, hp in enumerate(hps):
                    for ab in range(2):
                        nc.sync.dma_start(
                            out=sums16[2 * idx + ab:2 * idx + ab + 1, :],
                            in_=o_cmb[2 * hp + ab][64:65, :])
                sums4 = p3sb.tile([2 * n, T], F32, tag=f"s4{n}")
                if scalar_recip:
                    nc.scalar.activation(
                        out=sums4, in_=sums16,
                        func=mybir.ActivationFunctionType.Reciprocal,
                        scale=1.0)
                else:
                    nc.vector.tensor_copy(out=sums4, in_=sums16)
                    nc.vector.reciprocal(out=sums4, in_=sums4)
                for idx, hp in enumerate(hps):
                    nc.sync.dma_start(
                        out=recip_d[2 * hp:2 * hp + 2, :],
                        in_=sums4[2 * idx:2 * idx + 2, :])
                if mul_eng is None:
                    mul_eng = nc.vector
                for hp in hps:
                    for ab in range(2):
                        h = 2 * hp + ab
                        recipB = p3sb.tile([64, T], F32, tag="rb")
                        rd = recip_d[h:h + 1, :]
                        nc.sync.dma_start(out=recipB, in_=bass.AP(
                            tensor=rd.tensor, offset=rd.offset,
                            ap=[[0, 64]] + rd.ap[1:]))
                        mul_eng.tensor_mul(
                            out=attnT[hp][ab * 64:(ab + 1) * 64, :],
                            in0=o_cmb[h][0:64, :],
                            in1=recipB)

            for g in range(2):
                for hq in range(4):
                    attn_pass(g, hq, list(range(4)), drain_local)
            for hq in range(4):
                attn_pass(0, hq, rem_kts, drain_remote)
            for hq in range(4):
                attn_pass(1, hq, rem_kts, drain_remote)
                if hq == 0:
                    norm_group(0)
                if hq == 2:
                    norm_hps([4, 5], mul_eng=nc.gpsimd)
            norm_hps([6], mul_eng=nc.gpsimd)
            norm_hps([7])

        # ---------------- Phase 4: output projection ----------------
        # Split contraction it=0..6 / it=7: the first 56 matmuls only need
        # heads 0-13, so head pair 7's normalization chain (DVE + DMA
        # round-trip) hides under them; only the last 8 matmuls wait on it.
        with tc.tile_pool(name="p4sb", bufs=3) as p4sb, \
             tc.tile_pool(name="p4ps", bufs=1, space="PSUM") as p4ps:
            ps_y = {}
            for tt in range(NTT):
                for dc in range(2):
                    ps_y[tt, dc] = p4ps.tile([P, T], F32, tag=f"py{tt}_{dc}",
                                             name=f"py{tt}_{dc}")
            # it-major emission: consecutive matmuls hit different PSUM
            # banks, so the PE can overlap them instead of serializing on
            # one accumulator chain
            for it in range(NDT - 1):
                for tt in range(NTT):
                    for dc in range(2):
                        nc.tensor.matmul(
                            ps_y[tt, dc], attnT[it][:, tt * P:(tt + 1) * P],
                            wout_sb[:, it, dc * T:(dc + 1) * T],
                            start=(it == 0), stop=False)
            for tt in range(NTT):
                y_s = p4sb.tile([P, D], BF16, tag="y")
                for dc in range(2):
                    nc.tensor.matmul(
                        ps_y[tt, dc],
                        attnT[NDT - 1][:, tt * P:(tt + 1) * P],
                        wout_sb[:, NDT - 1, dc * T:(dc + 1) * T],
                        start=False, stop=True)
                    lo, hi = dc * T, (dc + 1) * T
                    if apply_b_out:
                        nc.vector.tensor_add(out=y_s[:, lo:hi],
                                             in0=ps_y[tt, dc],
                                             in1=boutB[:, lo:hi])
                    elif dc == 0:
                        nc.vector.tensor_copy(out=y_s[:, lo:hi],
                                              in_=ps_y[tt, dc])
                    else:
                        nc.scalar.copy(out=y_s[:, lo:hi], in_=ps_y[tt, dc])
                nc.sync.dma_start(
                    out=out_ext[tt * P:(tt + 1) * P, :], in_=y_s)

    _split_multiwaits(nc)
    return nc


_CACHE = {}
LAST_RESULTS = None


def kernel(x, ln_gamma, ln_beta, w_qkv, w_out, b_out):
    global LAST_RESULTS
    import ml_dtypes
    _maybe_install_ntff_hook()

    x = np.asarray(x, dtype=np.float32)
    ln_gamma = np.asarray(ln_gamma, dtype=np.float32).reshape(1, D)
    ln_beta = np.asarray(ln_beta, dtype=np.float32).reshape(1, D)
    w_qkv = np.asarray(w_qkv, dtype=np.float32)
    w_out = np.asarray(w_out, dtype=np.float32)
    b_out = np.asarray(b_out, dtype=np.float32).reshape(1, D)

    x16 = x.astype(ml_dtypes.bfloat16)
    w_qkv16 = np.ascontiguousarray(w_qkv.astype(ml_dtypes.bfloat16))
    w_o16 = np.ascontiguousarray(w_out.astype(ml_dtypes.bfloat16))

    apply_ln_affine = not (np.all(ln_gamma == 1.0) and np.all(ln_beta == 0.0))
    apply_b_out = not np.all(b_out == 0.0)

    key = (apply_ln_affine, apply_b_out)
    if key not in _CACHE:
        _CACHE[key] = build(*key)
    nc = _CACHE[key]

    in_maps = []
    for c in range(8):
        b, t = c // 4, c % 4
        in_maps.append({
            "x": np.ascontiguousarray(x16[b, t * T:(t + 1) * T, :]),
            "ln_gamma": ln_gamma,
            "ln_beta": ln_beta,
            "w_qkv16": w_qkv16,
            "w_o16": w_o16,
            "b_out": b_out,
        })

    trace = bool(os.environ.get("BASS_TRACE"))
    res = run_bass_kernel_spmd(nc, in_maps, core_ids=list(range(8)),
                               trace=trace)
    LAST_RESULTS = res

    y = np.empty((B, S, D), dtype=np.float32)
    for c in range(8):
        b, t = c // 4, c % 4
        y[b, t * T:(t + 1) * T, :] = np.asarray(
            res.results[c]["out"], dtype=np.float32)
    return y
